# revision 20
# baseline (speedup 1.0000x reference)
"""Grouped-expert SwiGLU kernel v3: straight-line static PE stream.

Tokens are tile-balanced across cores in MT-token slots. Expert weights are
selected per slot by a cond-predicated DMA with a dynamic DRAM offset: the
weights stay resident in a single SBUF buffer and are reloaded only at
slots where the expert changes (host precomputes per-slot (expert, load)
pairs in `meta`). No control flow in the device program, so DMA prefetch
and PE work pipeline freely across slots.

MT=256: the PE cost is proportional to streamed columns (power-throttled
clock), so finer slots cut padding waste (17*256=4352 vs 9*512=4608 token
capacity per core) at negligible per-matmul overhead.
"""

import math
import os

import ml_dtypes
import numpy as np

D = 2048
F = 512
MT = 256
TS = MT // 128
KC = D // 128
FC = F // 128
NCORES = 8
E = 8

_cache = {}


def _build(nt: int, smax: int, dyn_reps: bool = False,
           ablate: frozenset = frozenset()):
    import concourse.bacc as bacc
    import concourse.bass as bass
    import concourse.mybir as mybir
    from concourse.tile import TileContext

    dt = mybir.dt
    f32 = dt.float32
    bf16 = dt.bfloat16
    i32 = dt.int32
    AF = mybir.ActivationFunctionType

    nc = bacc.Bacc(
        "TRN2", target_bir_lowering=False, debug=False,
        enable_asserts=False, num_devices=NCORES,
    )

    XH = nc.dram_tensor("xh", [128, nt * KC * MT], bf16, kind="ExternalInput")
    W13T = nc.dram_tensor("w13t", [smax, 128, 2 * KC * F], bf16,
                          kind="ExternalInput")
    W2T = nc.dram_tensor("w2t", [smax, 128, FC * D], bf16,
                         kind="ExternalInput")
    META = nc.dram_tensor("meta", [1, 2 * nt + 1], i32, kind="ExternalInput")
    OUT = nc.dram_tensor("out", [nt * MT, D], bf16, kind="ExternalOutput")

    with TileContext(nc) as tc:
        with (
            tc.tile_pool(name="wp", bufs=1) as wp,
            tc.tile_pool(name="xp", bufs=6) as xp,
            tc.tile_pool(name="hp", bufs=3) as hp,
            tc.tile_pool(name="sl", bufs=4) as slp,
            tc.tile_pool(name="op", bufs=4) as op,
            tc.tile_pool(name="mp", bufs=1) as mp,
            tc.tile_pool(name="ps", bufs=8, space="PSUM") as ps,
        ):
            msb = mp.tile([1, 2 * nt + 1], i32, tag="meta")
            nc.sync.dma_start(out=msb[:], in_=META.ap())

            w13s = wp.tile([128, 2, KC, F], bf16, tag="w13s")
            w2s = wp.tile([128, FC, D], bf16, tag="w2s")

            evs, lvs = [], []
            for m in range(nt):
                evs.append(nc.snap(nc.values_load(
                    msb[0:1, 2 * m:2 * m + 1],
                    min_val=0, max_val=smax - 1,
                    skip_runtime_bounds_check=True)))
                lvs.append(nc.snap(nc.values_load(
                    msb[0:1, 2 * m + 1:2 * m + 2],
                    min_val=0, max_val=1,
                    skip_runtime_bounds_check=True)))

            # initial unconditional load of slot 0's expert; per-slot cond
            # loads then fire only where the host set the load flag
            nc.sync.dma_start(out=w13s[:], in_=W13T.ap()[bass.ds(evs[0], 1)])
            nc.sync.dma_start(out=w2s[:], in_=W2T.ap()[bass.ds(evs[0], 1)])

            def rep_body(first_rep: bool):
                for m in range(nt):
                    ev, lv = evs[m], lvs[m]
                    if "no_w" not in ablate:
                        nc.sync.dma_start(out=w13s[:],
                                          in_=W13T.ap()[bass.ds(ev, 1)],
                                          cond=lv, cond_hint=False)
                        nc.sync.dma_start(out=w2s[:],
                                          in_=W2T.ap()[bass.ds(ev, 1)],
                                          cond=lv, cond_hint=False)

                    xt = xp.tile([128, KC, MT], bf16, tag="xt")
                    if "no_x" not in ablate:
                        nc.sync.dma_start(
                            out=xt[:],
                            in_=XH[:, m * KC * MT:(m + 1) * KC * MT]
                            .rearrange("p (k t) -> p k t", k=KC))

                    use_ht = "no_act" not in ablate
                    if use_ht:
                        ht = hp.tile([128, FC, MT], bf16, tag="ht")
                    for f in range(FC):
                        x1b = ps.tile([128, 512], f32, tag="ps")
                        x3b = ps.tile([128, 512], f32, tag="ps")
                        x1t = x1b[:, 0:MT]
                        x3t = x3b[:, 0:MT]
                        for k in range(KC):
                            lhs1 = w13s[:, 0, k, f * 128:(f + 1) * 128]
                            lhs3 = w13s[:, 1, k, f * 128:(f + 1) * 128]
                            rhs = xt[:, k, :]
                            nc.tensor.matmul(x1t, lhs1, rhs,
                                             start=(k == 0), stop=(k == KC - 1))
                            nc.tensor.matmul(x3t, lhs3, rhs,
                                             start=(k == 0), stop=(k == KC - 1))
                        if use_ht:
                            sil = slp.tile([128, MT], f32, tag="sil")
                            nc.scalar.activation(sil[:], x1t, AF.Silu)
                            nc.vector.tensor_mul(ht[:, f, :], sil[:], x3t)

                    if "no_g2" in ablate:
                        continue
                    for ts in range(TS):
                        po = []
                        for _ in range(4):
                            pot = ps.tile([128, 512], f32, tag="ps")
                            po.append(pot)
                        for fc in range(FC):
                            lhs = (ht[:, fc, ts * 128:(ts + 1) * 128] if use_ht
                                   else w2s[:, fc, ts * 128:(ts + 1) * 128])
                            for dc in range(4):
                                nc.tensor.matmul(
                                    po[dc][:], lhs,
                                    w2s[:, fc, dc * 512:(dc + 1) * 512],
                                    start=(fc == 0), stop=(fc == FC - 1))
                        if "no_out" in ablate:
                            continue
                        osb = op.tile([128, D], bf16, tag="osb")
                        for dc in range(4):
                            nc.vector.tensor_copy(
                                osb[:, dc * 512:(dc + 1) * 512], po[dc][:])
                        nc.sync.dma_start(
                            out=OUT[m * MT + ts * 128:m * MT + (ts + 1) * 128, :],
                            in_=osb[:])

            if dyn_reps:
                repv = nc.snap(nc.values_load(
                    msb[0:1, 2 * nt:2 * nt + 1], min_val=1, max_val=4096,
                    skip_runtime_bounds_check=True))
                with tc.For_i(0, repv):
                    rep_body(False)
            else:
                rep_body(True)

    nc.compile()
    return nc


def _get_program(nt: int, smax: int, dyn_reps: bool = False,
                 ablate: frozenset = frozenset()):
    key = (nt, smax, dyn_reps, ablate)
    if key not in _cache:
        _cache[key] = _build(nt, smax, dyn_reps, ablate)
    return _cache[key]


def _assign(counts):
    """Greedy: chunk the padded-tile list into per-core runs of <=NT tiles
    spanning <=2 experts when possible. Returns (nt, per-core list of
    (expert, tile_lo, n_tiles) segments)."""
    En = len(counts)
    pt = [max(1, math.ceil(c / MT)) if c > 0 else 0 for c in counts]
    total = sum(pt)
    nt = math.ceil(total / NCORES)
    for nt_try in (nt, nt + 1):
        segs = [[] for _ in range(NCORES)]
        e, used = 0, 0
        for c in range(NCORES):
            cap = nt_try
            nexp = 0
            while cap > 0 and e < En:
                if pt[e] - used == 0:
                    e += 1
                    used = 0
                    continue
                if nexp == 2:
                    break
                take = min(cap, pt[e] - used)
                segs[c].append((e, used, take))
                used += take
                cap -= take
                nexp += 1
        leftover = total - sum(s[2] for core in segs for s in core)
        if leftover == 0:
            return nt_try, segs
    # fallback: linear chunking, any number of experts per core
    flat = []
    for e in range(En):
        flat += [e] * pt[e]
    nt = math.ceil(total / NCORES)
    segs = [[] for _ in range(NCORES)]
    for c in range(NCORES):
        chunk = flat[c * nt:(c + 1) * nt]
        i = 0
        while i < len(chunk):
            e = chunk[i]
            j = i
            while j < len(chunk) and chunk[j] == e:
                j += 1
            prior = flat[:c * nt].count(e)
            segs[c].append((e, prior, j - i))
            i = j
    return nt, segs


def _prepare(x, counts, w1, w2, w3):
    """Host-side sharding: returns (nt, smax, in_maps, placements)."""
    En = len(counts)
    starts = np.concatenate([[0], np.cumsum(counts)])[:En].astype(np.int64)

    nt, segs = _assign(counts)
    smax = max(2, max(len({s[0] for s in core}) for core in segs if core))

    bf = ml_dtypes.bfloat16
    # weights pre-transposed so the per-slot weight DMA is contiguous per
    # partition; w1 and w3 fused into one tensor (one cond-DMA per slot)
    w1t_full = np.ascontiguousarray(
        w1.astype(bf).reshape(En, KC, 128, F).transpose(0, 2, 1, 3)
    ).reshape(En, 128, KC * F)
    w3t_full = np.ascontiguousarray(
        w3.astype(bf).reshape(En, KC, 128, F).transpose(0, 2, 1, 3)
    ).reshape(En, 128, KC * F)
    w13t_full = np.ascontiguousarray(
        np.stack([w1t_full, w3t_full], axis=2)).reshape(En, 128, 2 * KC * F)
    w2t_full = np.ascontiguousarray(
        w2.astype(bf).reshape(En, FC, 128, D).transpose(0, 2, 1, 3)
    ).reshape(En, 128, FC * D)
    xb = x.astype(bf)

    in_maps = []
    placements = []  # per core: list of (slot, src_lo, nrows)
    for c in range(NCORES):
        cs = segs[c]
        exps = []
        for (e, _, _) in cs:
            if e not in exps:
                exps.append(e)
        emap = {e: i for i, e in enumerate(exps)}
        while len(exps) < smax:
            exps.append(exps[-1] if exps else 0)

        xh = np.zeros((128, nt, KC, MT), dtype=bf)
        meta = np.zeros((1, 2 * nt + 1), np.int32)
        meta[0, 2 * nt] = 1  # rep count (used only by the dyn_reps program)
        place = []
        slot = 0
        prev_e = None
        for (e, tile_lo, ntk) in cs:
            src_lo = int(starts[e]) + tile_lo * MT
            src_hi = min(int(starts[e]) + counts[e], src_lo + ntk * MT)
            for tk in range(ntk):
                lo = src_lo + tk * MT
                nrow = max(0, min(MT, src_hi - lo))
                if nrow > 0:
                    blk = xb[lo:lo + nrow].reshape(nrow, KC, 128)
                    xh[:, slot, :, :nrow] = blk.transpose(2, 1, 0)
                    place.append((slot, lo, nrow))
                meta[0, 2 * slot] = emap[e]
                # slot 0 is covered by the initial unconditional load
                meta[0, 2 * slot + 1] = 1 if (e != prev_e and slot > 0) else 0
                prev_e = e
                slot += 1
        while slot < nt:
            meta[0, 2 * slot] = emap[cs[-1][0]] if cs else 0
            meta[0, 2 * slot + 1] = 0
            slot += 1
        placements.append(place)
        in_maps.append({
            "xh": np.ascontiguousarray(xh.reshape(128, nt * KC * MT)),
            "w13t": np.ascontiguousarray(w13t_full[exps]),
            "w2t": np.ascontiguousarray(w2t_full[exps]),
            "meta": meta,
        })
    return nt, smax, in_maps, placements


def kernel(x, num_tokens_per_expert, w1, w2, w3):
    from concourse.bass_utils import run_bass_kernel_spmd

    x = np.asarray(x)
    counts = [int(v) for v in np.asarray(num_tokens_per_expert)]
    w1 = np.asarray(w1)
    w2 = np.asarray(w2)
    w3 = np.asarray(w3)
    T = x.shape[0]

    nt, smax, in_maps, placements = _prepare(x, counts, w1, w2, w3)
    nc = _get_program(nt, smax)

    res = run_bass_kernel_spmd(nc, in_maps, core_ids=list(range(NCORES)))
    kernel.last_results = res

    out = np.empty((T, D), dtype=np.float32)
    for c in range(NCORES):
        o = res.results[c]["out"]
        for (slot, src_lo, nrow) in placements[c]:
            out[src_lo:src_lo + nrow] = o[slot * MT:slot * MT + nrow].astype(
                np.float32)
    return out


# revision 25
# speedup vs baseline: 1.1375x; 1.1375x over previous
"""Grouped-expert SwiGLU kernel v3: straight-line static PE stream.

Tokens are tile-balanced across cores in MT-token slots. Expert weights are
selected per slot by a cond-predicated DMA with a dynamic DRAM offset: the
weights stay resident in a single SBUF buffer and are reloaded only at
slots where the expert changes (host precomputes per-slot (expert, load)
pairs in `meta`). No control flow in the device program, so DMA prefetch
and PE work pipeline freely across slots.

MT=512: in the single-shot (boost-clock) regime the per-matmul dispatch
overhead dominates over streamed columns, so the fewest, widest matmuls
win (N=512 is the ISA max per PSUM bank).
"""

import math
import os

import ml_dtypes
import numpy as np

D = 2048
F = 512
MT = 256
TS = MT // 128
KC = D // 128
FC = F // 128
NCORES = 8
E = 8

_cache = {}


def _build(nt: int, smax: int, dyn_reps: bool = False,
           ablate: frozenset = frozenset()):
    import concourse.bacc as bacc
    import concourse.bass as bass
    import concourse.mybir as mybir
    from concourse.tile import TileContext

    dt = mybir.dt
    f32 = dt.float32
    bf16 = dt.bfloat16
    i32 = dt.int32
    AF = mybir.ActivationFunctionType

    nc = bacc.Bacc(
        "TRN2", target_bir_lowering=False, debug=False,
        enable_asserts=False, num_devices=NCORES,
    )

    XH = nc.dram_tensor("xh", [128, nt * KC * MT], bf16, kind="ExternalInput")
    W13T = nc.dram_tensor("w13t", [smax, 128, 2 * KC * F], bf16,
                          kind="ExternalInput")
    W2T = nc.dram_tensor("w2t", [smax, 128, FC * D], bf16,
                         kind="ExternalInput")
    META = nc.dram_tensor("meta", [1, 2 * nt + 1], i32, kind="ExternalInput")
    OUT = nc.dram_tensor("out", [nt * MT, D], bf16, kind="ExternalOutput")

    with TileContext(nc) as tc:
        with (
            tc.tile_pool(name="wp", bufs=1) as wp,
            tc.tile_pool(name="xp", bufs=6) as xp,
            tc.tile_pool(name="hp", bufs=3) as hp,
            tc.tile_pool(name="sl", bufs=4) as slp,
            tc.tile_pool(name="op", bufs=4) as op,
            tc.tile_pool(name="mp", bufs=1) as mp,
            tc.tile_pool(name="ps", bufs=8, space="PSUM") as ps,
        ):
            msb = mp.tile([1, 2 * nt + 1], i32, tag="meta")
            nc.sync.dma_start(out=msb[:], in_=META.ap())

            w13s = wp.tile([128, 2, KC, F], bf16, tag="w13s")
            w2s = wp.tile([128, FC, D], bf16, tag="w2s")

            evs, lvs = [], []
            for m in range(nt):
                evs.append(nc.snap(nc.values_load(
                    msb[0:1, 2 * m:2 * m + 1],
                    min_val=0, max_val=smax - 1,
                    skip_runtime_bounds_check=True)))
                lvs.append(nc.snap(nc.values_load(
                    msb[0:1, 2 * m + 1:2 * m + 2],
                    min_val=0, max_val=1,
                    skip_runtime_bounds_check=True)))

            # initial unconditional load of slot 0's expert; per-slot cond
            # loads then fire only where the host set the load flag
            nc.sync.dma_start(out=w13s[:], in_=W13T.ap()[bass.ds(evs[0], 1)])
            nc.sync.dma_start(out=w2s[:], in_=W2T.ap()[bass.ds(evs[0], 1)])

            def rep_body(first_rep: bool):
                for m in range(nt):
                    ev, lv = evs[m], lvs[m]
                    if "no_w" not in ablate:
                        nc.sync.dma_start(out=w13s[:],
                                          in_=W13T.ap()[bass.ds(ev, 1)],
                                          cond=lv, cond_hint=False)
                        nc.sync.dma_start(out=w2s[:],
                                          in_=W2T.ap()[bass.ds(ev, 1)],
                                          cond=lv, cond_hint=False)

                    # x/out ride the ACT HWDGE ring so the SP ring's
                    # cond-weight-DMA WAR waits never block them
                    xt = xp.tile([128, KC, MT], bf16, tag="xt")
                    if "no_x" not in ablate:
                        nc.scalar.dma_start(
                            out=xt[:],
                            in_=XH[:, m * KC * MT:(m + 1) * KC * MT]
                            .rearrange("p (k t) -> p k t", k=KC))

                    use_ht = "no_act" not in ablate
                    if use_ht:
                        ht = hp.tile([128, FC, MT], bf16, tag="ht")
                    for f in range(FC):
                        x1b = ps.tile([128, 512], f32, tag="ps")
                        x3b = ps.tile([128, 512], f32, tag="ps")
                        x1t = x1b[:, 0:MT]
                        x3t = x3b[:, 0:MT]
                        for k in range(KC):
                            lhs1 = w13s[:, 0, k, f * 128:(f + 1) * 128]
                            lhs3 = w13s[:, 1, k, f * 128:(f + 1) * 128]
                            rhs = xt[:, k, :]
                            nc.tensor.matmul(x1t, lhs1, rhs,
                                             start=(k == 0), stop=(k == KC - 1))
                            nc.tensor.matmul(x3t, lhs3, rhs,
                                             start=(k == 0), stop=(k == KC - 1))
                        if use_ht:
                            sil = slp.tile([128, MT], f32, tag="sil")
                            nc.scalar.activation(sil[:], x1t, AF.Silu)
                            nc.vector.tensor_mul(ht[:, f, :], sil[:], x3t)

                    if "no_g2" in ablate:
                        continue
                    for ts in range(TS):
                        po = []
                        for _ in range(4):
                            pot = ps.tile([128, 512], f32, tag="ps")
                            po.append(pot)
                        for fc in range(FC):
                            lhs = (ht[:, fc, ts * 128:(ts + 1) * 128] if use_ht
                                   else w2s[:, fc, ts * 128:(ts + 1) * 128])
                            for dc in range(4):
                                nc.tensor.matmul(
                                    po[dc][:], lhs,
                                    w2s[:, fc, dc * 512:(dc + 1) * 512],
                                    start=(fc == 0), stop=(fc == FC - 1))
                        if "no_out" in ablate:
                            continue
                        osb = op.tile([128, D], bf16, tag="osb")
                        for dc in range(4):
                            nc.vector.tensor_copy(
                                osb[:, dc * 512:(dc + 1) * 512], po[dc][:])
                        nc.scalar.dma_start(
                            out=OUT[m * MT + ts * 128:m * MT + (ts + 1) * 128, :],
                            in_=osb[:])

            if dyn_reps:
                repv = nc.snap(nc.values_load(
                    msb[0:1, 2 * nt:2 * nt + 1], min_val=1, max_val=4096,
                    skip_runtime_bounds_check=True))
                with tc.For_i(0, repv):
                    rep_body(False)
            else:
                rep_body(True)

    nc.compile()
    return nc


def _get_program(nt: int, smax: int, dyn_reps: bool = False,
                 ablate: frozenset = frozenset()):
    key = (nt, smax, dyn_reps, ablate)
    if key not in _cache:
        _cache[key] = _build(nt, smax, dyn_reps, ablate)
    return _cache[key]


def _assign(counts):
    """Greedy: chunk the padded-tile list into per-core runs of <=NT tiles
    spanning <=2 experts when possible. Returns (nt, per-core list of
    (expert, tile_lo, n_tiles) segments)."""
    En = len(counts)
    pt = [max(1, math.ceil(c / MT)) if c > 0 else 0 for c in counts]
    total = sum(pt)
    nt = math.ceil(total / NCORES)
    for nt_try in (nt, nt + 1):
        segs = [[] for _ in range(NCORES)]
        e, used = 0, 0
        for c in range(NCORES):
            cap = nt_try
            nexp = 0
            while cap > 0 and e < En:
                if pt[e] - used == 0:
                    e += 1
                    used = 0
                    continue
                if nexp == 2:
                    break
                take = min(cap, pt[e] - used)
                segs[c].append((e, used, take))
                used += take
                cap -= take
                nexp += 1
        leftover = total - sum(s[2] for core in segs for s in core)
        if leftover == 0:
            return nt_try, segs
    # fallback: linear chunking, any number of experts per core
    flat = []
    for e in range(En):
        flat += [e] * pt[e]
    nt = math.ceil(total / NCORES)
    segs = [[] for _ in range(NCORES)]
    for c in range(NCORES):
        chunk = flat[c * nt:(c + 1) * nt]
        i = 0
        while i < len(chunk):
            e = chunk[i]
            j = i
            while j < len(chunk) and chunk[j] == e:
                j += 1
            prior = flat[:c * nt].count(e)
            segs[c].append((e, prior, j - i))
            i = j
    return nt, segs


def _prepare(x, counts, w1, w2, w3):
    """Host-side sharding: returns (nt, smax, in_maps, placements)."""
    En = len(counts)
    starts = np.concatenate([[0], np.cumsum(counts)])[:En].astype(np.int64)

    nt, segs = _assign(counts)
    smax = max(2, max(len({s[0] for s in core}) for core in segs if core))

    bf = ml_dtypes.bfloat16
    # weights pre-transposed so the per-slot weight DMA is contiguous per
    # partition; w1 and w3 fused into one tensor (one cond-DMA per slot)
    w1t_full = np.ascontiguousarray(
        w1.astype(bf).reshape(En, KC, 128, F).transpose(0, 2, 1, 3)
    ).reshape(En, 128, KC * F)
    w3t_full = np.ascontiguousarray(
        w3.astype(bf).reshape(En, KC, 128, F).transpose(0, 2, 1, 3)
    ).reshape(En, 128, KC * F)
    w13t_full = np.ascontiguousarray(
        np.stack([w1t_full, w3t_full], axis=2)).reshape(En, 128, 2 * KC * F)
    w2t_full = np.ascontiguousarray(
        w2.astype(bf).reshape(En, FC, 128, D).transpose(0, 2, 1, 3)
    ).reshape(En, 128, FC * D)
    xb = x.astype(bf)

    in_maps = []
    placements = []  # per core: list of (slot, src_lo, nrows)
    for c in range(NCORES):
        cs = segs[c]
        exps = []
        for (e, _, _) in cs:
            if e not in exps:
                exps.append(e)
        emap = {e: i for i, e in enumerate(exps)}
        while len(exps) < smax:
            exps.append(exps[-1] if exps else 0)

        xh = np.zeros((128, nt, KC, MT), dtype=bf)
        meta = np.zeros((1, 2 * nt + 1), np.int32)
        meta[0, 2 * nt] = 1  # rep count (used only by the dyn_reps program)
        place = []
        slot = 0
        prev_e = None
        for (e, tile_lo, ntk) in cs:
            src_lo = int(starts[e]) + tile_lo * MT
            src_hi = min(int(starts[e]) + counts[e], src_lo + ntk * MT)
            for tk in range(ntk):
                lo = src_lo + tk * MT
                nrow = max(0, min(MT, src_hi - lo))
                if nrow > 0:
                    blk = xb[lo:lo + nrow].reshape(nrow, KC, 128)
                    xh[:, slot, :, :nrow] = blk.transpose(2, 1, 0)
                    place.append((slot, lo, nrow))
                meta[0, 2 * slot] = emap[e]
                # slot 0 is covered by the initial unconditional load
                meta[0, 2 * slot + 1] = 1 if (e != prev_e and slot > 0) else 0
                prev_e = e
                slot += 1
        while slot < nt:
            meta[0, 2 * slot] = emap[cs[-1][0]] if cs else 0
            meta[0, 2 * slot + 1] = 0
            slot += 1
        placements.append(place)
        in_maps.append({
            "xh": np.ascontiguousarray(xh.reshape(128, nt * KC * MT)),
            "w13t": np.ascontiguousarray(w13t_full[exps]),
            "w2t": np.ascontiguousarray(w2t_full[exps]),
            "meta": meta,
        })
    return nt, smax, in_maps, placements


def kernel(x, num_tokens_per_expert, w1, w2, w3):
    from concourse.bass_utils import run_bass_kernel_spmd

    x = np.asarray(x)
    counts = [int(v) for v in np.asarray(num_tokens_per_expert)]
    w1 = np.asarray(w1)
    w2 = np.asarray(w2)
    w3 = np.asarray(w3)
    T = x.shape[0]

    nt, smax, in_maps, placements = _prepare(x, counts, w1, w2, w3)
    nc = _get_program(nt, smax)

    res = run_bass_kernel_spmd(nc, in_maps, core_ids=list(range(NCORES)))
    kernel.last_results = res

    out = np.empty((T, D), dtype=np.float32)
    for c in range(NCORES):
        o = res.results[c]["out"]
        for (slot, src_lo, nrow) in placements[c]:
            out[src_lo:src_lo + nrow] = o[slot * MT:slot * MT + nrow].astype(
                np.float32)
    return out
